# revision 21
# baseline (speedup 1.0000x reference)
"""Farthest-point-sampling Bass kernel for Trainium2 (8 NeuronCores, data-parallel over batch).

Contract: kernel(xyz, npoint) -> (new_xyz, idx) matching reference FPS bit-exactly.
Each core handles 2 of the 16 batches; the 1024-step FPS scan runs fully on-chip:
  - dist update: ScalarE Square activations (bias = -centroid), coordinate sum via
    identity-matmul PSUM accumulation on TensorE, then one fused DVE
    tensor_tensor_reduce = min-update + per-partition max in a single pass
  - argmax: max_index (per-partition first-occurrence); the cross-partition winner
    comes from a PE transpose + free-dim reduce of the key mask*(BIG-n), which
    resolves ties to the lowest global index exactly like jnp.argmax
  - centroid gather: winner column index loaded into a register -> dynamic-slice copy
    of the interleaved xyz tile, winner row picked by a one-hot PE matmul, then a
    ones-matmul broadcast feeds the next step's activation biases
"""

import functools

import numpy as np

P = 128
B_FULL, N_FULL, NPOINT_FULL = 16, 131072, 1024
NCORES, BPC = 8, 2  # cores, batches per core
UNROLL = 4
BIG = float(1 << 24)
MMF = 512           # max matmul free dim (one PSUM bank of fp32)

# start = jax.random.randint(jax.random.fold_in(jax.random.key(0), 1), (16,), 0, 131072)
# (fixed seed in reference.py; hardcoded so kernel.py needs no jax at build time)
START = [67342, 109190, 107272, 41711, 121131, 39767, 20256, 111054,
         6031, 2016, 37726, 102408, 83259, 45101, 19925, 61549]


def build_fps_program(n, npoint, unroll=UNROLL):
    """Build the Bass program for BPC batches of n points, npoint samples."""
    import concourse.bacc as bacc
    import concourse.bass as bass
    import concourse.mybir as mybir
    from concourse.ordered_set import OrderedSet
    from concourse.tile import TileContext

    dt = mybir.dt
    Alu = mybir.AluOpType
    Act = mybir.ActivationFunctionType

    f = n // P          # free-dim length per partition (power of two)
    assert f * P == n and (f & (f - 1)) == 0

    nc = bacc.Bacc(trn_type="TRN2", num_devices=NCORES)
    xyz_in = nc.dram_tensor("xyz_in", [BPC, n, 3], dt.float32, kind="ExternalInput")
    init_bias = nc.dram_tensor("init_bias", [P, 3 * BPC], dt.float32, kind="ExternalInput")
    init_enc = nc.dram_tensor("init_enc", [1, BPC], dt.float32, kind="ExternalInput")
    idn_in = nc.dram_tensor("idn_in", [P, P], dt.float32, kind="ExternalInput")
    ones_in = nc.dram_tensor("ones_in", [1, P], dt.float32, kind="ExternalInput")
    pbmb_in = nc.dram_tensor("pbmb_in", [P, 1], dt.float32, kind="ExternalInput")
    idx_out = nc.dram_tensor("idx_out", [BPC, npoint], dt.int32, kind="ExternalOutput")

    with TileContext(nc) as tc:
        with tc.tile_pool(name="main", bufs=1) as pool, \
             tc.tile_pool(name="ps", bufs=1, space="PSUM") as pspool:
            def til(nm, shape, dty):
                return [pool.tile(shape, dty, tag=f"{nm}{b}", name=f"{nm}{b}")
                        for b in range(BPC)]

            xyzI = til("xyzI", [P, 3 * f], dt.float32)
            xP = til("xP", [P, f], dt.float32)
            yP = til("yP", [P, f], dt.float32)
            zP = til("zP", [P, f], dt.float32)
            dist = til("dist", [P, f], dt.float32)
            sqx = til("sqx", [P, f], dt.float32)
            sqy = til("sqy", [P, f], dt.float32)
            sqz = til("sqz", [P, f], dt.float32)
            maxv8 = til("maxv8", [P, 8], dt.float32)
            idx8 = til("idx8", [P, 8], dt.uint16)
            mask = til("mask", [P, 1], dt.float32)
            nbig = til("nbig", [P, 1], dt.float32)
            km = til("km", [P, 1], dt.float32)
            rowoh = til("rowoh", [P, 1], dt.float32)
            menc = til("menc", [1, 2], dt.float32)   # [M, enc*] on partition 0
            nsti = til("nsti", [1, 1], dt.int32)
            fi3 = til("fi3", [1, 1], dt.int32)
            t3 = til("t3", [P, 3], dt.float32)
            bneg = til("bneg", [1, 3], dt.float32)
            biasT = til("biasT", [P, 3], dt.float32)
            encst = til("encst", [1, npoint], dt.float32)
            encring = til("encring", [1, unroll], dt.float32)
            idxi32 = til("idxi32", [1, npoint], dt.int32)
            idn = pool.tile([P, P], dt.float32, tag="idn", name="idn")
            ones = pool.tile([1, P], dt.float32, tag="ones", name="ones")
            pbmb = pool.tile([P, 1], dt.float32, tag="pbmb", name="pbmb")
            s2p = [pspool.tile([P, f], dt.float32, tag=f"s2p{b}", name=f"s2p{b}")
                   for b in range(BPC)]
            # one PSUM bank per batch for all the small PE outputs:
            # cols 0:128 and 128:256 transpose scratch, 256 Mb, 257 encb,
            # 258:261 biasp, 264:267 c (partition 0)
            scr = [pspool.tile([P, MMF], dt.float32, tag=f"scr{b}", name=f"scr{b}")
                   for b in range(BPC)]

            # ---------- init ----------
            nc.sync.dma_start(idn[:], idn_in[:, :])
            nc.sync.dma_start(ones[:], ones_in[:, :])
            nc.sync.dma_start(pbmb[:], pbmb_in[:, :])
            for b in range(BPC):
                nc.sync.dma_start(
                    xyzI[b][:],
                    xyz_in[b].rearrange("(p q) c -> p (q c)", p=P),
                )
                src3 = xyzI[b][:].rearrange("p (q c) -> p c q", c=3)
                nc.vector.tensor_copy(xP[b][:], src3[:, 0, :])
                nc.scalar.copy(yP[b][:], src3[:, 1, :])
                nc.scalar.copy(zP[b][:], src3[:, 2, :])
                nc.vector.memset(dist[b][:], 1e10)
                nc.vector.memset(maxv8[b][:], -3.0e38)
                nc.sync.dma_start(biasT[b][:], init_bias[:, 3 * b:3 * b + 3])
                nc.sync.dma_start(encst[b][0:1, 0:1], init_enc[0:1, b:b + 1])

            def step(b, ring_u=None, tail_col=None):
                # enc result goes to encring[ring_u] (loop body) or
                # encst[tail_col] (static tail)
                nc.scalar.activation(sqx[b][:], xP[b][:], Act.Square,
                                     bias=biasT[b][:, 0:1])
                nc.scalar.activation(sqy[b][:], yP[b][:], Act.Square,
                                     bias=biasT[b][:, 1:2])
                nc.scalar.activation(sqz[b][:], zP[b][:], Act.Square,
                                     bias=biasT[b][:, 2:3])
                # s2 = (sqx + sqy) + sqz, accumulated in PSUM by identity matmuls
                for h in range(0, f, MMF):
                    sl = slice(h, min(h + MMF, f))
                    nc.tensor.matmul(s2p[b][:, sl], idn[:], sqx[b][:, sl],
                                     start=True, stop=False)
                    nc.tensor.matmul(s2p[b][:, sl], idn[:], sqy[b][:, sl],
                                     start=False, stop=False)
                    nc.tensor.matmul(s2p[b][:, sl], idn[:], sqz[b][:, sl],
                                     start=False, stop=True)
                nc.vector.tensor_tensor(dist[b][:], dist[b][:], s2p[b][:],
                                        Alu.min)
                nc.vector.tensor_reduce(maxv8[b][:, 0:1], dist[b][:],
                                        axis=mybir.AxisListType.X, op=Alu.max)
                # global max M: PE transpose of per-partition maxima + reduce
                nc.tensor.transpose(scr[b][0:1, 0:P], maxv8[b][:, 0:1], idn[:])
                nc.vector.tensor_reduce(menc[b][0:1, 0:1], scr[b][0:1, 0:P],
                                        axis=mybir.AxisListType.X, op=Alu.max)
                nc.vector.max_index(idx8[b][:], maxv8[b][:], dist[b][:])
                # broadcast M to all partitions, build winner key mask*(BIG-n)
                nc.tensor.matmul(scr[b][:, 256:257], ones[:], menc[b][0:1, 0:1],
                                 start=True, stop=True)
                nc.vector.tensor_tensor(mask[b][:], maxv8[b][:, 0:1],
                                        scr[b][:, 256:257], Alu.is_equal)
                nc.vector.scalar_tensor_tensor(nbig[b][:], idx8[b][:, 0:1], -1.0,
                                               pbmb[:], Alu.mult, Alu.add)
                nc.vector.tensor_mul(km[b][:], mask[b][:], nbig[b][:])
                # global winner enc* = max over partitions of km
                nc.tensor.transpose(scr[b][0:1, 128:256], km[b][:], idn[:])
                nc.vector.tensor_reduce(menc[b][0:1, 1:2], scr[b][0:1, 128:256],
                                        axis=mybir.AxisListType.X, op=Alu.max)
                nc.tensor.matmul(scr[b][:, 257:258], ones[:], menc[b][0:1, 1:2],
                                 start=True, stop=True)
                nc.vector.tensor_tensor(rowoh[b][:], km[b][:],
                                        scr[b][:, 257:258], Alu.is_equal)
                # winner flat index -> 3*(n mod f) on partition 0
                nc.vector.tensor_scalar(nsti[b][:], menc[b][0:1, 1:2], -1.0, BIG,
                                        Alu.mult, Alu.add)
                nc.vector.tensor_scalar(fi3[b][:], nsti[b][:], f - 1, None,
                                        Alu.bitwise_and)
                nc.vector.tensor_scalar(fi3[b][:], fi3[b][:], 3, None, Alu.mult)
                # gather the winner's coords: dynamic column slice + one-hot row pick
                geng = nc.scalar if b == 0 else nc.vector
                reg = geng.alloc_register(f"off{b}_{nc.next_id()}")
                geng.reg_load(reg, fi3[b][0:1, 0:1])
                off = nc.snap(reg, donate=True, min_val=0, max_val=3 * (f - 1))
                if b == 0:
                    nc.scalar.copy(t3[b][:], xyzI[b][:, bass.ds(off, 3)])
                else:
                    nc.vector.tensor_copy(t3[b][:], xyzI[b][:, bass.ds(off, 3)])
                nc.tensor.matmul(scr[b][0:1, 264:267], rowoh[b][:], t3[b][:],
                                 start=True, stop=True)
                nc.scalar.activation(bneg[b][:], scr[b][0:1, 264:267], Act.Copy,
                                     scale=-1.0)
                # broadcast -c to all partitions for the next step's biases
                nc.tensor.matmul(scr[b][:, 258:261], ones[:], bneg[b][:],
                                 start=True, stop=True)
                nc.vector.tensor_copy(biasT[b][:], scr[b][:, 258:261])
                if tail_col is not None:
                    nc.scalar.copy(encst[b][0:1, tail_col:tail_col + 1],
                                   menc[b][0:1, 1:2])
                else:
                    nc.scalar.copy(encring[b][0:1, ring_u:ring_u + 1],
                                   menc[b][0:1, 1:2])

            # ---------- main loop: steps 1..npoint-1 ----------
            nsteps = npoint - 1
            nloops = nsteps // unroll
            if nloops > 0:
                with tc.For_i(0, nloops) as i:
                    for u in range(unroll):
                        for b in range(BPC):
                            step(b, ring_u=u)
                    col = nc.snap(i * unroll + 1, min_val=1,
                                  max_val=npoint - unroll,
                                  engines=OrderedSet(
                                      [mybir.EngineType.Activation]))
                    for b in range(BPC):
                        nc.scalar.copy(encst[b][0:1, bass.ds(col, unroll)],
                                       encring[b][:])
            for t in range(nloops * unroll + 1, npoint):
                for b in range(BPC):
                    step(b, tail_col=t)

            # ---------- outputs ----------
            for b in range(BPC):
                nc.vector.tensor_scalar(idxi32[b][:], encst[b][:], -1.0, BIG,
                                        Alu.mult, Alu.add)
                nc.sync.dma_start(idx_out[b].rearrange("(a q) -> a q", a=1),
                                  idxi32[b][:])

    nc.compile()
    return nc


@functools.lru_cache(maxsize=2)
def _compiled_program(n, npoint, unroll):
    return build_fps_program(n, npoint, unroll)


def make_core_inputs(xyz, start, n=N_FULL):
    """Shard full xyz (B,n,3) into per-core input maps."""
    idn = np.eye(P, dtype=np.float32)
    ones = np.ones((1, P), dtype=np.float32)
    pbmb = (BIG - (n // P) * np.arange(P, dtype=np.float64)).astype(np.float32)[:, None]
    in_maps = []
    ncores = xyz.shape[0] // BPC
    for c in range(ncores):
        bs = list(range(c * BPC, (c + 1) * BPC))
        bias = np.zeros((P, 3 * BPC), dtype=np.float32)
        en = np.zeros((1, BPC), dtype=np.float32)
        for j, gb in enumerate(bs):
            s = int(start[gb])
            bias[:, 3 * j:3 * j + 3] = -xyz[gb, s]
            en[0, j] = BIG - s
        in_maps.append({
            "xyz_in": np.ascontiguousarray(xyz[bs[0]:bs[-1] + 1], dtype=np.float32),
            "init_bias": bias,
            "init_enc": en,
            "idn_in": idn,
            "ones_in": ones,
            "pbmb_in": pbmb,
        })
    return in_maps


def run_on_hw(nc, in_maps, **kwargs):
    """Run with callback/trap instructions stripped (the HW can't encode them)."""
    from concourse.bass_interp import get_hw_module
    from concourse.bass_utils import run_bass_kernel_spmd

    old_m = nc.m
    nc.m = get_hw_module(nc.m)
    try:
        return run_bass_kernel_spmd(nc, in_maps,
                                    core_ids=list(range(len(in_maps))), **kwargs)
    finally:
        nc.m = old_m


def kernel(xyz, npoint):
    xyz = np.asarray(xyz, dtype=np.float32)
    npoint = int(npoint)
    assert xyz.shape == (B_FULL, N_FULL, 3) and npoint == NPOINT_FULL

    nc = _compiled_program(N_FULL, NPOINT_FULL, UNROLL)
    in_maps = make_core_inputs(xyz, START)
    res = run_on_hw(nc, in_maps)
    idx = np.concatenate([r["idx_out"] for r in res.results], axis=0).astype(np.int32)
    new_xyz = xyz[np.arange(B_FULL)[:, None], idx]
    return new_xyz, idx
